# revision 4
# baseline (speedup 1.0000x reference)
"""Trainium2 Bass kernel for CohereAttention (GQA + interleaved RoPE + causal).

Sharding: DP-2 over batch x TP-4 over heads (8 NeuronCores).
Core i handles batch b=i//4 and head-group t=i%4 (8 q heads, 2 kv heads).
Each core computes a partial output projection; the host sums the 4 TP
partials per batch (host-side all-reduce).

Compute is bf16 on the TensorEngine (fp32 PSUM accumulation), feature-major
layouts throughout:
  GEMM1:  qkv^T [1536, T]  = w_qkv_shard^T-tiles @ hidden^T
  RoPE:   rot(x) via constant 128x128 permutation matmul + cos/sin tables
  Attn:   S^T [k,q] = K^T-tile.T @ Q^T ; exp on ACT; PV and row-sum on PE
  GEMM2:  out^T [4096, T] partial = w_o-tiles.T @ attn^T
"""

import numpy as np
import ml_dtypes

import concourse.bass as bass
import concourse.tile as tile
from concourse import bacc, mybir
from concourse.bass_utils import run_bass_kernel_spmd
from concourse.masks import make_identity

BF16 = ml_dtypes.bfloat16

B, S, H = 2, 2048, 4096
NH, NKV, D = 32, 8, 128
G = NH // NKV
THETA = 10000.0

# per-core shard sizes (DP2 x TP4)
QH = 8           # q heads per core
KH = 2           # kv heads per core
N1 = (QH + 2 * KH) * D   # 1536 qkv columns per core
AO = QH * D      # 1024 attn-out dims per core
TB = 512         # token block
NTB = S // TB    # 4 blocks
NKT = S // 128   # 16 k tiles of 128 tokens
SCALE = float(D) ** -0.5


def build_nc():
    nc = bacc.Bacc("TRN2", target_bir_lowering=False, debug=False,
                   enable_asserts=False)
    dt = mybir.dt

    hidT = nc.dram_tensor("hidT", [H, S], dt.bfloat16, kind="ExternalInput").ap()
    wq_t = nc.dram_tensor("wq_t", [N1 // 128, 128, H], dt.bfloat16,
                          kind="ExternalInput").ap()
    wo_t = nc.dram_tensor("wo_t", [H // 128, 128, AO], dt.bfloat16,
                          kind="ExternalInput").ap()
    cosE = nc.dram_tensor("cosE", [128, S], dt.bfloat16, kind="ExternalInput").ap()
    sinE = nc.dram_tensor("sinE", [128, S], dt.bfloat16, kind="ExternalInput").ap()
    maskd = nc.dram_tensor("maskd", [128, 4 * TB], dt.bfloat16,
                           kind="ExternalInput").ap()
    rotmd = nc.dram_tensor("rotmd", [128, 128], dt.bfloat16,
                           kind="ExternalInput").ap()
    outT = nc.dram_tensor("outT", [H, S], dt.bfloat16, kind="ExternalOutput").ap()

    with tile.TileContext(nc) as tc:
        with (
            tc.tile_pool(name="const", bufs=1) as const,
            tc.tile_pool(name="persist", bufs=1) as persist,
            tc.tile_pool(name="hid", bufs=1) as hid_pool,
            tc.tile_pool(name="wq", bufs=3) as wq_pool,
            tc.tile_pool(name="pre", bufs=1) as pre_pool,
            tc.tile_pool(name="qrope", bufs=1) as qrope_pool,
            tc.tile_pool(name="attnT", bufs=1) as attnT_pool,
            tc.tile_pool(name="probs", bufs=6) as probs_pool,
            tc.tile_pool(name="wo", bufs=4) as wo_pool,
            tc.tile_pool(name="tmp", bufs=2) as tmp_pool,
            tc.tile_pool(name="norm", bufs=2) as norm_pool,
            tc.tile_pool(name="ost", bufs=4) as ost_pool,
            tc.tile_pool(name="mm_ps", bufs=2, space="PSUM") as mm_ps,
            tc.tile_pool(name="tp_ps", bufs=2, space="PSUM") as tp_ps,
            tc.tile_pool(name="s_ps", bufs=2, space="PSUM") as s_ps_pool,
            tc.tile_pool(name="o_ps", bufs=1, space="PSUM") as o_ps_pool,
            tc.tile_pool(name="r_ps", bufs=1, space="PSUM") as r_ps_pool,
        ):
            # constants
            ident = const.tile([128, 128], dt.bfloat16)
            make_identity(nc, ident)
            ones = const.tile([128, 1], dt.bfloat16)
            nc.any.memset(ones[:], 1.0)
            rotm = const.tile([128, 128], dt.bfloat16)
            nc.sync.dma_start(rotm[:], rotmd)
            masks = const.tile([128, 4 * TB], dt.bfloat16)
            nc.sync.dma_start(masks[:], maskd)
            cos_sb = const.tile([128, S], dt.bfloat16)
            nc.sync.dma_start(cos_sb[:], cosE)
            sin_sb = const.tile([128, S], dt.bfloat16)
            nc.sync.dma_start(sin_sb[:], sinE)

            # persistent K^T (feature-major) and V (token-major) caches
            KTp = [persist.tile([128, S], dt.bfloat16, tag=f"KT{i}", name=f"KT{i}")
                   for i in range(KH)]
            Vp = [persist.tile([128, S], dt.bfloat16, tag=f"V{i}", name=f"V{i}")
                  for i in range(KH)]

            for tb in range(NTB):
                tsl = bass.ts(tb, TB)
                # ---- load hidden^T block [32 x [128, TB]] ----
                hid = []
                for k in range(H // 128):
                    t = hid_pool.tile([128, TB], dt.bfloat16, tag=f"hid{k}")
                    nc.sync.dma_start(t[:], hidT[k * 128:(k + 1) * 128, tsl])
                    hid.append(t)

                # ---- GEMM1: qkv^T block, m-tiles of 128 ----
                pre = []
                for m in range(N1 // 128):
                    wsl = wq_pool.tile([128, H], dt.bfloat16)
                    nc.sync.dma_start(wsl[:], wq_t[m])
                    ps = mm_ps.tile([128, TB], dt.float32)
                    for k in range(H // 128):
                        nc.tensor.matmul(ps[:], wsl[:, k * 128:(k + 1) * 128],
                                         hid[k][:],
                                         start=(k == 0), stop=(k == H // 128 - 1))
                    pt = pre_pool.tile([128, TB], dt.bfloat16, tag=f"pre{m}")
                    nc.scalar.copy(pt[:], ps[:])
                    pre.append(pt)

                # ---- RoPE on q (8) and k (2) tiles ----
                qT = []
                for idx in range(QH + KH):
                    src = pre[idx]
                    rps = mm_ps.tile([128, TB], dt.float32, tag="ps")
                    nc.tensor.matmul(rps[:], rotm[:], src[:], start=True, stop=True)
                    t1 = tmp_pool.tile([128, TB], dt.float32, tag="ropet1")
                    nc.vector.tensor_mul(t1[:], src[:], cos_sb[:, tsl])
                    t2 = tmp_pool.tile([128, TB], dt.float32, tag="ropet2")
                    nc.vector.tensor_mul(t2[:], rps[:], sin_sb[:, tsl])
                    if idx < QH:
                        dst = qrope_pool.tile([128, TB], dt.bfloat16, tag=f"q{idx}")
                        nc.vector.tensor_add(dst[:], t1[:], t2[:])
                        qT.append(dst)
                    else:
                        nc.vector.tensor_add(KTp[idx - QH][:, tsl], t1[:], t2[:])

                # ---- V^T -> V (token-major) via PE transpose ----
                for kvh in range(KH):
                    vsrc = pre[QH + KH + kvh]
                    for tt in range(TB // 128):
                        tp = tp_ps.tile([128, 128], dt.bfloat16)
                        nc.tensor.transpose(tp[:], vsrc[:, tt * 128:(tt + 1) * 128],
                                            ident[:])
                        kt_g = tb * 4 + tt
                        nc.scalar.copy(Vp[kvh][:, kt_g * 128:(kt_g + 1) * 128], tp[:])

                # ---- attention for this q block ----
                attnT = []
                nkt = 4 * (tb + 1)
                for h in range(QH):
                    kvh = h // G
                    ops = o_ps_pool.tile([128, TB], dt.float32)
                    rps_ = r_ps_pool.tile([1, TB], dt.float32)
                    for kt in range(nkt):
                        sps = s_ps_pool.tile([128, TB], dt.float32)
                        nc.tensor.matmul(sps[:], KTp[kvh][:, kt * 128:(kt + 1) * 128],
                                         qT[h][:], start=True, stop=True)
                        pr = probs_pool.tile([128, TB], dt.bfloat16, tag="probs")
                        nc.scalar.activation(pr[:], sps[:],
                                             mybir.ActivationFunctionType.Exp,
                                             scale=SCALE)
                        if kt >= 4 * tb:
                            v = kt - 4 * tb
                            nc.vector.tensor_mul(pr[:], pr[:],
                                                 masks[:, v * TB:(v + 1) * TB])
                        nc.tensor.matmul(ops[:], Vp[kvh][:, kt * 128:(kt + 1) * 128],
                                         pr[:], start=(kt == 0), stop=(kt == nkt - 1))
                        nc.tensor.matmul(rps_[:], ones[:, 0:1], pr[:],
                                         start=(kt == 0), stop=(kt == nkt - 1))
                    rs = norm_pool.tile([1, TB], dt.float32, tag="rs")
                    nc.vector.tensor_copy(rs[:], rps_[:])
                    rc = norm_pool.tile([1, TB], dt.float32, tag="rc")
                    nc.vector.reciprocal(rc[:], rs[:])
                    rb = norm_pool.tile([128, TB], dt.float32, tag="rb")
                    nc.gpsimd.partition_broadcast(rb[:], rc[:])
                    at = attnT_pool.tile([128, TB], dt.bfloat16, tag=f"at{h}")
                    nc.vector.tensor_mul(at[:], ops[:], rb[:])
                    attnT.append(at)

                # ---- GEMM2: partial out^T block ----
                for mh in range(H // 128):
                    wos = wo_pool.tile([128, AO], dt.bfloat16)
                    nc.sync.dma_start(wos[:], wo_t[mh])
                    ps = mm_ps.tile([128, TB], dt.float32)
                    for a in range(AO // 128):
                        nc.tensor.matmul(ps[:], wos[:, a * 128:(a + 1) * 128],
                                         attnT[a][:],
                                         start=(a == 0), stop=(a == AO // 128 - 1))
                    ot = ost_pool.tile([128, TB], dt.bfloat16)
                    nc.scalar.copy(ot[:], ps[:])
                    nc.sync.dma_start(outT[mh * 128:(mh + 1) * 128, tsl], ot[:])

    nc.compile()
    return nc


_NC_CACHE = []


def _get_nc():
    if not _NC_CACHE:
        _NC_CACHE.append(build_nc())
    return _NC_CACHE[0]


def make_host_inputs(hidden_states, positions, w_qkv, w_o):
    """Build per-core input maps (8 cores: core i -> batch i//4, head grp i%4)."""
    inv_freq = 1.0 / (THETA ** (np.arange(0, D, 2, dtype=np.float64) / D))

    # constant tensors (same for all cores)
    rotm = np.zeros((128, 128), np.float32)
    for i in range(64):
        rotm[2 * i, 2 * i + 1] = 1.0   # lhsT = R^T
        rotm[2 * i + 1, 2 * i] = -1.0
    rotm = rotm.astype(BF16)

    masks = np.zeros((128, 4 * TB), np.float32)
    j = np.arange(TB)
    for v in range(4):
        ii = np.arange(128)
        masks[:, v * TB:(v + 1) * TB] = (j[None, :] >= (ii[:, None] + 128 * v))
    masks = masks.astype(BF16)

    in_maps = []
    for core in range(8):
        b, t = core // 4, core % 4
        # qkv weight shard: 8 q heads, 2 kv heads (k then v)
        qc = w_qkv[:, 8 * t * D:(8 * t + 8) * D]
        kc = w_qkv[:, NH * D + 2 * t * D: NH * D + (2 * t + 2) * D]
        vc = w_qkv[:, (NH + NKV) * D + 2 * t * D: (NH + NKV) * D + (2 * t + 2) * D]
        wshard = np.concatenate([qc, kc, vc], axis=1).astype(BF16)  # [H, N1]
        wq_t = np.ascontiguousarray(
            wshard.reshape(H // 128, 128, N1 // 128, 128)
            .transpose(2, 1, 0, 3).reshape(N1 // 128, 128, H))

        wo_shard = w_o[AO * t:AO * (t + 1), :].astype(BF16)  # [AO, H]
        wo_t = np.ascontiguousarray(
            wo_shard.reshape(AO // 128, 128, H // 128, 128)
            .transpose(2, 1, 0, 3).reshape(H // 128, 128, AO))

        hidT = np.ascontiguousarray(hidden_states[b].T).astype(BF16)  # [H, S]

        pos = positions[b].astype(np.float64)          # [S]
        freqs = pos[:, None] * inv_freq[None, :]       # [S, 64]
        cosE = np.repeat(np.cos(freqs).T, 2, axis=0).astype(BF16)  # [128, S]
        sinE = np.repeat(np.sin(freqs).T, 2, axis=0).astype(BF16)

        in_maps.append({
            "hidT": hidT, "wq_t": wq_t, "wo_t": wo_t,
            "cosE": cosE, "sinE": sinE, "maskd": masks, "rotmd": rotm,
        })
    return in_maps


def combine_outputs(results):
    out = np.zeros((B, S, H), np.float32)
    for core in range(8):
        b = core // 4
        out[b] += results[core]["outT"].astype(np.float32).T
    return out


def kernel(hidden_states, positions, w_qkv, w_o):
    hidden_states = np.asarray(hidden_states, dtype=np.float32)
    positions = np.asarray(positions)
    w_qkv = np.asarray(w_qkv, dtype=np.float32)
    w_o = np.asarray(w_o, dtype=np.float32)

    nc = _get_nc()
    in_maps = make_host_inputs(hidden_states, positions, w_qkv, w_o)
    res = run_bass_kernel_spmd(nc, in_maps, core_ids=list(range(8)))
    return combine_outputs(res.results)


# revision 7
# speedup vs baseline: 7684.2383x; 7684.2383x over previous
"""Trainium2 Bass kernel for CohereAttention (GQA + interleaved RoPE + causal).

Sharding: DP-2 over batch x TP-4 over heads (8 NeuronCores).
Core i handles batch b=i//4 and head-group t=i%4 (8 q heads, 2 kv heads).
Each core computes a partial output projection; the host sums the 4 TP
partials per batch (host-side all-reduce).

Compute is bf16 on the TensorEngine (fp32 PSUM accumulation), feature-major
layouts throughout:
  GEMM1:  qkv^T [1536, T]  = w_qkv_shard^T-tiles @ hidden^T
  RoPE:   rot(x) via constant 128x128 permutation matmul + cos/sin tables
  Attn:   S^T [k,q] = K^T-tile.T @ Q^T ; exp on ACT; PV and row-sum on PE
  GEMM2:  out^T [4096, T] partial = w_o-tiles.T @ attn^T
"""

import numpy as np
import ml_dtypes

import concourse.bass as bass
import concourse.tile as tile
from concourse import bacc, mybir
from concourse.bass_utils import run_bass_kernel_spmd
from concourse.masks import make_identity

BF16 = ml_dtypes.bfloat16

B, S, H = 2, 2048, 4096
NH, NKV, D = 32, 8, 128
G = NH // NKV
THETA = 10000.0

# per-core shard sizes (DP2 x TP4)
QH = 8           # q heads per core
KH = 2           # kv heads per core
N1 = (QH + 2 * KH) * D   # 1536 qkv columns per core
AO = QH * D      # 1024 attn-out dims per core
TB = 512         # token block
NTB = S // TB    # 4 blocks
NKT = S // 128   # 16 k tiles of 128 tokens
SCALE = float(D) ** -0.5


def build_nc(reps=1):
    nc = bacc.Bacc("TRN2", target_bir_lowering=False, debug=False,
                   enable_asserts=False)
    dt = mybir.dt

    hidT = nc.dram_tensor("hidT", [H, S], dt.bfloat16, kind="ExternalInput").ap()
    wq_t = nc.dram_tensor("wq_t", [N1 // 128, 128, H], dt.bfloat16,
                          kind="ExternalInput").ap()
    wo_t = nc.dram_tensor("wo_t", [H // 128, 128, AO], dt.bfloat16,
                          kind="ExternalInput").ap()
    cosE = nc.dram_tensor("cosE", [128, S], dt.bfloat16, kind="ExternalInput").ap()
    sinE = nc.dram_tensor("sinE", [128, S], dt.bfloat16, kind="ExternalInput").ap()
    maskd = nc.dram_tensor("maskd", [128, TB], dt.bfloat16,
                           kind="ExternalInput").ap()
    rotmd = nc.dram_tensor("rotmd", [128, 128], dt.bfloat16,
                           kind="ExternalInput").ap()
    outT = nc.dram_tensor("outT", [H, S], dt.bfloat16, kind="ExternalOutput").ap()

    with tile.TileContext(nc) as tc:
        with (
            tc.tile_pool(name="const", bufs=1) as const,
            tc.tile_pool(name="persist", bufs=1) as persist,
            tc.tile_pool(name="hid", bufs=1) as hid_pool,
            tc.tile_pool(name="wq", bufs=3) as wq_pool,
            tc.tile_pool(name="pre", bufs=1) as pre_pool,
            tc.tile_pool(name="qrope", bufs=1) as qrope_pool,
            tc.tile_pool(name="attnT", bufs=1) as attnT_pool,
            tc.tile_pool(name="probs", bufs=6) as probs_pool,
            tc.tile_pool(name="wo", bufs=4) as wo_pool,
            tc.tile_pool(name="tmp", bufs=2) as tmp_pool,
            tc.tile_pool(name="norm", bufs=2) as norm_pool,
            tc.tile_pool(name="ost", bufs=4) as ost_pool,
            tc.tile_pool(name="mm_ps", bufs=2, space="PSUM") as mm_ps,
            tc.tile_pool(name="tp_ps", bufs=1, space="PSUM") as tp_ps,
            tc.tile_pool(name="s_ps", bufs=2, space="PSUM") as s_ps_pool,
            tc.tile_pool(name="o_ps", bufs=2, space="PSUM") as o_ps_pool,
            tc.tile_pool(name="r_ps", bufs=1, space="PSUM") as r_ps_pool,
        ):
            # constants
            ident = const.tile([128, 128], dt.bfloat16)
            make_identity(nc, ident)
            ones = const.tile([128, 1], dt.float32)
            nc.any.memset(ones[:], 1.0)
            rotm = const.tile([128, 128], dt.bfloat16)
            nc.sync.dma_start(rotm[:], rotmd)
            masks = const.tile([128, TB], dt.bfloat16)
            nc.sync.dma_start(masks[:], maskd)
            cos_sb = const.tile([128, S], dt.bfloat16)
            nc.sync.dma_start(cos_sb[:], cosE)
            sin_sb = const.tile([128, S], dt.bfloat16)
            nc.sync.dma_start(sin_sb[:], sinE)

            # persistent K^T (feature-major) and V (token-major) caches
            KTp = [persist.tile([128, S], dt.bfloat16, tag=f"KT{i}", name=f"KT{i}")
                   for i in range(KH)]
            Vp = [persist.tile([128, S], dt.bfloat16, tag=f"V{i}", name=f"V{i}")
                  for i in range(KH)]

            rep_ctx = tc.For_i(0, reps, 1) if reps > 1 else None
            if rep_ctx is not None:
                rep_ctx.__enter__()
            for tb in range(NTB):
                tsl = bass.ts(tb, TB)
                # ---- load hidden^T block [32 x [128, TB]] ----
                hid = []
                for k in range(H // 128):
                    t = hid_pool.tile([128, TB], dt.bfloat16, tag=f"hid{k}")
                    nc.sync.dma_start(t[:], hidT[k * 128:(k + 1) * 128, tsl])
                    hid.append(t)

                # ---- GEMM1: qkv^T block, m-tiles of 128 ----
                pre = []
                for m in range(N1 // 128):
                    wsl = wq_pool.tile([128, H], dt.bfloat16)
                    nc.sync.dma_start(wsl[:], wq_t[m])
                    ps = mm_ps.tile([128, TB], dt.float32)
                    for k in range(H // 128):
                        nc.tensor.matmul(ps[:], wsl[:, k * 128:(k + 1) * 128],
                                         hid[k][:],
                                         start=(k == 0), stop=(k == H // 128 - 1))
                    pt = pre_pool.tile([128, TB], dt.bfloat16, tag=f"pre{m}")
                    nc.scalar.copy(pt[:], ps[:])
                    pre.append(pt)

                # ---- RoPE on q (8) and k (2) tiles ----
                qT = []
                for idx in range(QH + KH):
                    src = pre[idx]
                    rps = mm_ps.tile([128, TB], dt.float32, tag="ps")
                    nc.tensor.matmul(rps[:], rotm[:], src[:], start=True, stop=True)
                    t1 = tmp_pool.tile([128, TB], dt.float32, tag="ropet1")
                    nc.vector.tensor_mul(t1[:], src[:], cos_sb[:, tsl])
                    t2 = tmp_pool.tile([128, TB], dt.float32, tag="ropet2")
                    nc.vector.tensor_mul(t2[:], rps[:], sin_sb[:, tsl])
                    if idx < QH:
                        dst = qrope_pool.tile([128, TB], dt.bfloat16, tag=f"q{idx}")
                        nc.vector.tensor_add(dst[:], t1[:], t2[:])
                        qT.append(dst)
                    else:
                        nc.vector.tensor_add(KTp[idx - QH][:, tsl], t1[:], t2[:])

                # ---- V^T -> V (token-major) via PE transpose ----
                for kvh in range(KH):
                    vsrc = pre[QH + KH + kvh]
                    for tt in range(TB // 128):
                        tp = tp_ps.tile([128, 128], dt.bfloat16)
                        nc.tensor.transpose(tp[:], vsrc[:, tt * 128:(tt + 1) * 128],
                                            ident[:])
                        kt_g = tb * 4 + tt
                        nc.scalar.copy(Vp[kvh][:, kt_g * 128:(kt_g + 1) * 128], tp[:])

                # ---- attention for this q block ----
                attnT = []
                nkt = 4 * (tb + 1)
                for h in range(QH):
                    kvh = h // G
                    ops = o_ps_pool.tile([128, TB], dt.float32)
                    rps_ = r_ps_pool.tile([1, TB], dt.float32)
                    racc = norm_pool.tile([128, TB], dt.float32, tag="racc")
                    for kt in range(nkt):
                        v = kt - 4 * tb
                        q0 = 128 * v if v > 0 else 0
                        N = TB - q0
                        sps = s_ps_pool.tile([128, TB], dt.float32)
                        nc.tensor.matmul(sps[:, :N],
                                         KTp[kvh][:, kt * 128:(kt + 1) * 128],
                                         qT[h][:, q0:TB], start=True, stop=True)
                        pr = probs_pool.tile([128, TB], dt.bfloat16, tag="probs")
                        nc.scalar.activation(pr[:, :N], sps[:, :N],
                                             mybir.ActivationFunctionType.Exp,
                                             scale=SCALE)
                        if v >= 0:
                            nc.vector.tensor_mul(pr[:, :N], pr[:, :N],
                                                 masks[:, :N])
                        nc.tensor.matmul(ops[:, q0:TB],
                                         Vp[kvh][:, kt * 128:(kt + 1) * 128],
                                         pr[:, :N], start=(kt == 0),
                                         stop=(kt == nkt - 1))
                        if kt == 0:
                            nc.vector.tensor_copy(racc[:], pr[:, :N])
                        else:
                            nc.vector.tensor_add(racc[:, q0:TB], racc[:, q0:TB],
                                                 pr[:, :N])
                    nc.tensor.matmul(rps_[:], ones[:].bitcast(dt.float32r),
                                     racc[:].bitcast(dt.float32r),
                                     start=True, stop=True)
                    rs = norm_pool.tile([1, TB], dt.float32, tag="rs")
                    nc.vector.tensor_copy(rs[:], rps_[:])
                    rc = norm_pool.tile([1, TB], dt.float32, tag="rc")
                    nc.vector.reciprocal(rc[:], rs[:])
                    rb = norm_pool.tile([128, TB], dt.float32, tag="rb")
                    nc.gpsimd.partition_broadcast(rb[:], rc[:])
                    at = attnT_pool.tile([128, TB], dt.bfloat16, tag=f"at{h}")
                    nc.vector.tensor_mul(at[:], ops[:], rb[:])
                    attnT.append(at)

                # ---- GEMM2: partial out^T block ----
                for mh in range(H // 128):
                    wos = wo_pool.tile([128, AO], dt.bfloat16)
                    nc.sync.dma_start(wos[:], wo_t[mh])
                    ps = mm_ps.tile([128, TB], dt.float32)
                    for a in range(AO // 128):
                        nc.tensor.matmul(ps[:], wos[:, a * 128:(a + 1) * 128],
                                         attnT[a][:],
                                         start=(a == 0), stop=(a == AO // 128 - 1))
                    ot = ost_pool.tile([128, TB], dt.bfloat16)
                    nc.scalar.copy(ot[:], ps[:])
                    nc.sync.dma_start(outT[mh * 128:(mh + 1) * 128, tsl], ot[:])

            if rep_ctx is not None:
                rep_ctx.__exit__(None, None, None)

    nc.compile()
    return nc


build_nc_reps = True


_NC_CACHE = []


def _get_nc():
    if not _NC_CACHE:
        _NC_CACHE.append(build_nc())
    return _NC_CACHE[0]


def make_host_inputs(hidden_states, positions, w_qkv, w_o):
    """Build per-core input maps (8 cores: core i -> batch i//4, head grp i%4)."""
    inv_freq = 1.0 / (THETA ** (np.arange(0, D, 2, dtype=np.float64) / D))

    # constant tensors (same for all cores)
    rotm = np.zeros((128, 128), np.float32)
    for i in range(64):
        rotm[2 * i, 2 * i + 1] = 1.0   # lhsT = R^T
        rotm[2 * i + 1, 2 * i] = -1.0
    rotm = rotm.astype(BF16)

    masks = np.zeros((128, TB), np.float32)
    j = np.arange(TB)
    ii = np.arange(128)
    masks[:, :] = (j[None, :] >= ii[:, None])
    masks = masks.astype(BF16)

    in_maps = []
    for core in range(8):
        b, t = core // 4, core % 4
        # qkv weight shard: 8 q heads, 2 kv heads (k then v)
        qc = w_qkv[:, 8 * t * D:(8 * t + 8) * D]
        kc = w_qkv[:, NH * D + 2 * t * D: NH * D + (2 * t + 2) * D]
        vc = w_qkv[:, (NH + NKV) * D + 2 * t * D: (NH + NKV) * D + (2 * t + 2) * D]
        wshard = np.concatenate([qc, kc, vc], axis=1).astype(BF16)  # [H, N1]
        wq_t = np.ascontiguousarray(
            wshard.reshape(H // 128, 128, N1 // 128, 128)
            .transpose(2, 1, 0, 3).reshape(N1 // 128, 128, H))

        wo_shard = w_o[AO * t:AO * (t + 1), :].astype(BF16)  # [AO, H]
        wo_t = np.ascontiguousarray(
            wo_shard.reshape(AO // 128, 128, H // 128, 128)
            .transpose(2, 1, 0, 3).reshape(H // 128, 128, AO))

        hidT = np.ascontiguousarray(hidden_states[b].T).astype(BF16)  # [H, S]

        pos = positions[b].astype(np.float64)          # [S]
        freqs = pos[:, None] * inv_freq[None, :]       # [S, 64]
        cosE = np.repeat(np.cos(freqs).T, 2, axis=0).astype(BF16)  # [128, S]
        sinE = np.repeat(np.sin(freqs).T, 2, axis=0).astype(BF16)

        in_maps.append({
            "hidT": hidT, "wq_t": wq_t, "wo_t": wo_t,
            "cosE": cosE, "sinE": sinE, "maskd": masks, "rotmd": rotm,
        })
    return in_maps


def combine_outputs(results):
    out = np.zeros((B, S, H), np.float32)
    for core in range(8):
        b = core // 4
        out[b] += results[core]["outT"].astype(np.float32).T
    return out


def kernel(hidden_states, positions, w_qkv, w_o):
    hidden_states = np.asarray(hidden_states, dtype=np.float32)
    positions = np.asarray(positions)
    w_qkv = np.asarray(w_qkv, dtype=np.float32)
    w_o = np.asarray(w_o, dtype=np.float32)

    nc = _get_nc()
    in_maps = make_host_inputs(hidden_states, positions, w_qkv, w_o)
    res = run_bass_kernel_spmd(nc, in_maps, core_ids=list(range(8)))
    return combine_outputs(res.results)
